# revision 2
# baseline (speedup 1.0000x reference)
"""Trainium2 Bass kernel for the CIGAR GNN message-passing model (v2).

Data-parallel over batch across 8 NeuronCores (512 rows/core). All bulk
gathers use the custom dma_gather ucode (256B rows, int16 shard-local
indices, 4 SWDGE queues). Host builds compacted (order-preserving) index
streams per (batch-tile, table-shard), padded so segment sums become
selector matmuls with statically-known structure; overflow rows spill to
one per-partition DIRECT2D gather per batch-tile.

  - adgroup seq-sum: bf16 rows padded to 128 elems; 4 shards; streams
    grouped by 32-batch windows (13 tiles each) -> [32,32] PSUM windows.
  - cate seq-sum: count-matmul (exact bf16 integer counts x bf16 table).
  - GNN: bf16 mem0|mem1 256B rows; 7 shards; per-tile PE transpose ->
    blockdiag(W0^T,W1^T) matmul -> tanh -> transpose -> selector matmul
    with mask*0.5/len folded into the bf16 selector values.
  - user/item singles: [P,1] indirect DMA (DIRECT2D), exact f32.
  - MLP on transposed [320, 512] features in f32.
"""

import numpy as np

import concourse.bass as bass
import concourse.bacc as bacc
import concourse.mybir as mybir
import concourse.tile as tile
from concourse.bass_utils import run_bass_kernel_spmd
from concourse.masks import make_identity

NC = 8
B, S, N, D, G = 4096, 200, 64, 32, 64
BC = B // NC  # 512
NBT = BC // 128  # 4
V0, V1, VM = 100001, 10001, 200000
SHA = 25088  # adgroup shard width (int16 range, uniform load)
SHG = 28672  # mem shard width
NSH_A = 4  # adgroup table shards
NSH_G = 7  # mem table shards
LW = 1664  # adgroup rows per (window, shard) block: 13 tiles
TW = LW // 128  # 13
NW = 4  # 32-batch windows per bt
LA = LW * NW  # 6656 rows per (bt, shard) stream
TA = LA // 128  # 52
LG = 1280  # GNN rows per (bt, shard) stream: 10 tiles
TG = LG // 128
V1P = 79 * 128  # padded cate rows (10112)
NQ = 4  # SWDGE queues

F32 = mybir.dt.float32
BF16 = mybir.dt.bfloat16
I16 = mybir.dt.int16
I32 = mybir.dt.int32

DEBUG = False
import os as _os
PARTS = _os.environ.get("KPARTS", "aucgm")  # adgroup,user,cate,gnn,mlp-spills
_CACHE = {}


def _build():
    nc = bacc.Bacc(None, target_bir_lowering=False, num_swdge_queues=NQ)

    # ---- DRAM inputs ----
    tab0b = nc.dram_tensor("tab0b", [NSH_A * SHA, 128], BF16, kind="ExternalInput")
    mem01b = nc.dram_tensor("mem01b", [NSH_G * SHG, 128], BF16, kind="ExternalInput")
    tab0f = nc.dram_tensor("tab0f", [V0, D], F32, kind="ExternalInput")
    tab1f = nc.dram_tensor("tab1f", [V1, D], F32, kind="ExternalInput")
    ut0 = nc.dram_tensor("ut0", [50000, D], F32, kind="ExternalInput")
    ut1 = nc.dram_tensor("ut1", [50000, D], F32, kind="ExternalInput")
    tab1b = nc.dram_tensor("tab1b", [V1P, D], BF16, kind="ExternalInput")
    counts = nc.dram_tensor("counts", [NBT * 79 * 128, 128], BF16, kind="ExternalInput")
    iu0 = nc.dram_tensor("iu0", [BC], I32, kind="ExternalInput")
    iu1 = nc.dram_tensor("iu1", [BC], I32, kind="ExternalInput")
    iad = nc.dram_tensor("iad", [BC], I32, kind="ExternalInput")
    icat = nc.dram_tensor("icat", [BC], I32, kind="ExternalInput")
    aidx = nc.dram_tensor("aidx", [NBT * NSH_A * 128, LA // 16], I16, kind="ExternalInput")
    gidx = nc.dram_tensor("gidx", [NBT * NSH_G * 128, LG // 16], I16, kind="ExternalInput")
    asel = nc.dram_tensor("asel", [NBT * NSH_A * 128, TA * 32], BF16, kind="ExternalInput")
    gsel = nc.dram_tensor("gsel", [NBT * NSH_G * 128, TG * 128], BF16, kind="ExternalInput")
    as_idx = nc.dram_tensor("as_idx", [NBT * 128], I32, kind="ExternalInput")
    as_sel = nc.dram_tensor("as_sel", [NBT * 128, 128], BF16, kind="ExternalInput")
    gs_idx = nc.dram_tensor("gs_idx", [NBT * 128], I32, kind="ExternalInput")
    gs_sel = nc.dram_tensor("gs_sel", [NBT * 128, 128], BF16, kind="ExternalInput")
    invseq = nc.dram_tensor("invseq", [128, NBT], F32, kind="ExternalInput")
    wstack = nc.dram_tensor("wstack", [128, 128], BF16, kind="ExternalInput")
    bstack = nc.dram_tensor("bstack", [128], F32, kind="ExternalInput")
    w1t = nc.dram_tensor("w1t", [320, 256], F32, kind="ExternalInput")
    b1d = nc.dram_tensor("b1", [256], F32, kind="ExternalInput")
    w2t = nc.dram_tensor("w2t", [256, 128], F32, kind="ExternalInput")
    b2d = nc.dram_tensor("b2", [128], F32, kind="ExternalInput")
    w3t = nc.dram_tensor("w3t", [128, 1], F32, kind="ExternalInput")
    b3d = nc.dram_tensor("b3", [1], F32, kind="ExternalInput")
    out = nc.dram_tensor("out", [BC], F32, kind="ExternalOutput")
    dbg = {}
    if DEBUG:
        for nm in ("dbgU", "dbgI", "dbgM", "dbgG"):
            dbg[nm] = nc.dram_tensor(nm, [BC, 64], F32, kind="ExternalOutput")

    qrr = [0]

    def q():  # round-robin SWDGE queue picker
        qrr[0] = (qrr[0] + 1) % NQ
        return qrr[0]

    def gnn_tile(nc, src_ap, sel_ap, gacc, identb, wst, bst, xp, pch, k, start, stop):
        xt_ps = pch.tile([128, 128], BF16, tag="pchain", name=f"x{k % 4}")
        nc.tensor.transpose(out=xt_ps[:], in_=src_ap, identity=identb[:])
        xt = xp.tile([128, 128], BF16, tag="xt", name=f"xt{k % 4}")
        nc.vector.tensor_copy(out=xt[:], in_=xt_ps[:])
        ht_ps = pch.tile([128, 128], F32, tag="pchain", name=f"h{k % 4}")
        nc.tensor.matmul(ht_ps[:], lhsT=wst[:], rhs=xt[:])
        hts = xp.tile([128, 128], BF16, tag="hts", name=f"ht{k % 4}")
        nc.scalar.activation(
            out=hts[:], in_=ht_ps[:],
            func=mybir.ActivationFunctionType.Tanh, bias=bst[:, 0:1],
        )
        h_ps = pch.tile([128, 128], BF16, tag="pchain", name=f"hp{k % 4}")
        nc.tensor.transpose(out=h_ps[:], in_=hts[:], identity=identb[:])
        hh = xp.tile([128, 128], BF16, tag="hh", name=f"hh{k % 4}")
        nc.vector.tensor_copy(out=hh[:], in_=h_ps[:])
        nc.tensor.matmul(gacc[:], lhsT=sel_ap, rhs=hh[:], start=start, stop=stop)

    with tile.TileContext(nc) as tc:
        with (
            tc.tile_pool(name="const", bufs=1) as cpool,
            tc.tile_pool(name="sb", bufs=2) as sb,
            tc.tile_pool(name="ag", bufs=5) as agp,
            tc.tile_pool(name="gg", bufs=6) as ggp,
            tc.tile_pool(name="sel", bufs=5) as selp,
            tc.tile_pool(name="cnt", bufs=3) as cntp,
            tc.tile_pool(name="x", bufs=4) as xp,
            tc.tile_pool(name="pch", bufs=4, space="PSUM") as pch,
            tc.tile_pool(name="pga", bufs=2, space="PSUM") as pga,
            tc.tile_pool(name="pms", bufs=2, space="PSUM") as pms,
        ):
            # ---- constants ----
            identb = cpool.tile([128, 128], BF16)
            make_identity(nc, identb[:])
            identf = cpool.tile([128, 128], F32)
            make_identity(nc, identf[:])
            wst = cpool.tile([128, 128], BF16)
            nc.sync.dma_start(out=wst[:], in_=wstack[:])
            bst = cpool.tile([128, 1], F32)
            nc.sync.dma_start(out=bst[:], in_=bstack[:, None])
            invs = cpool.tile([128, NBT], F32)
            nc.sync.dma_start(out=invs[:], in_=invseq[:])
            w1ts = [cpool.tile([128, 256], F32, tag=f"w1t{k}", name=f"w1t{k}") for k in range(3)]
            for k in range(3):
                lo, hi = k * 128, min((k + 1) * 128, 320)
                nc.sync.dma_start(out=w1ts[k][: hi - lo, :], in_=w1t[lo:hi, :])
            w2ts = [cpool.tile([128, 128], F32, tag=f"w2t{k}", name=f"w2t{k}") for k in range(2)]
            for k in range(2):
                nc.sync.dma_start(out=w2ts[k][:], in_=w2t[k * 128 : (k + 1) * 128, :])
            w3ts = cpool.tile([128, 1], F32)
            nc.sync.dma_start(out=w3ts[:], in_=w3t[:])
            b1s = [cpool.tile([128, 1], F32, tag=f"b1{k}", name=f"b1{k}") for k in range(2)]
            for k in range(2):
                nc.sync.dma_start(out=b1s[k][:], in_=b1d[k * 128 : (k + 1) * 128, None])
            b2s = cpool.tile([128, 1], F32)
            nc.sync.dma_start(out=b2s[:], in_=b2d[:, None])
            b3s = cpool.tile([1, 1], F32)
            nc.sync.dma_start(out=b3s[:], in_=b3d[:, None])
            t1res = cpool.tile([128, 79 * D], BF16)
            if "c" in PARTS:
                nc.sync.dma_start(
                    out=t1res[:].rearrange("p (c f) -> p c f", c=79),
                    in_=tab1b[:]
                    .rearrange("(c p) f -> c p f", p=128)
                    .transpose([1, 0, 2]),
                )
            fT = [
                cpool.tile([128, BC], F32, tag="fT0", name="fT0"),
                cpool.tile([128, BC], F32, tag="fT1", name="fT1"),
                cpool.tile([64, BC], F32, tag="fT2", name="fT2"),
            ]

            for bt in range(NBT):
                bsl = slice(bt * 128, (bt + 1) * 128)

                # ---- user / item embeddings (exact f32 DIRECT2D) ----
                U = sb.tile([128, 2 * D], F32, tag="U")
                I = sb.tile([128, 2 * D], F32, tag="I")
                if "u" not in PARTS:
                    nc.vector.memset(U[:], 0.0)
                    nc.vector.memset(I[:], 0.0)
                for dst, col, idxd, tabd in (
                    (U, 0, iu0, ut0),
                    (U, D, iu1, ut1),
                    (I, 0, iad, tab0f),
                    (I, D, icat, tab1f),
                ) if "u" in PARTS else ():
                    it = sb.tile([128, 1], I32, tag="smallidx")
                    nc.sync.dma_start(out=it[:], in_=idxd[bsl, None])
                    nc.gpsimd.indirect_dma_start(
                        out=dst[:, col : col + D],
                        out_offset=None,
                        in_=tabd[:],
                        in_offset=bass.IndirectOffsetOnAxis(ap=it[:], axis=0),
                    )

                # ---- adgroup seq-sum ----
                M = sb.tile([128, 2 * D], F32, tag="M")
                if "a" not in PARTS:
                    nc.vector.memset(M[:], 0.0)
                adests, asels = ([], []) if "a" in PARTS else (None, None)
                for sh in range(NSH_A if "a" in PARTS else 0):
                    r0 = (bt * NSH_A + sh) * 128
                    it = sb.tile([128, LA // 16], I16, tag="aidx")
                    nc.sync.dma_start(out=it[:], in_=aidx[r0 : r0 + 128, :])
                    dest = agp.tile([128, TA * 128], BF16, tag="adest")
                    nc.gpsimd.dma_gather(
                        out_ap=dest[:].rearrange("p (s e) -> p s e", e=128),
                        in_ap=tab0b[sh * SHA : (sh + 1) * SHA, :],
                        idxs_ap=it[:],
                        num_idxs=LA,
                        num_idxs_reg=LA,
                        elem_size=128,
                        single_packet=False,
                        queue_num=q(),
                    )
                    sl = selp.tile([128, TA * 32], BF16, tag="asel")
                    nc.sync.dma_start(out=sl[:], in_=asel[r0 : r0 + 128, :])
                    adests.append(dest)
                    asels.append(sl)
                for w in range(NW if "a" in PARTS else 0):
                    wps = pms.tile([32, 32], F32, tag="pmisc", name=f"wps{bt}_{w}")
                    k = 0
                    for sh in range(NSH_A):
                        for j in range(w * TW, (w + 1) * TW):
                            nc.tensor.matmul(
                                wps[:],
                                lhsT=asels[sh][:, j * 32 : (j + 1) * 32],
                                rhs=adests[sh][:, j * 128 : j * 128 + 32],
                                start=(k == 0),
                                stop=(k == NSH_A * TW - 1),
                            )
                            k += 1
                    nc.vector.tensor_copy(out=M[32 * w : 32 * w + 32, :D], in_=wps[:])
                # adgroup spill
                if "a" in PARTS:
                    sit = sb.tile([128, 1], I32, tag="spidx")
                    nc.sync.dma_start(out=sit[:], in_=as_idx[bsl, None])
                    sga = sb.tile([128, D], F32, tag="sga")
                    nc.gpsimd.indirect_dma_start(
                        out=sga[:], out_offset=None, in_=tab0f[:],
                        in_offset=bass.IndirectOffsetOnAxis(ap=sit[:], axis=0),
                    )
                    sgab = sb.tile([128, D], BF16, tag="sgab")
                    nc.vector.tensor_copy(out=sgab[:], in_=sga[:])
                    ssel = sb.tile([128, 128], BF16, tag="ssel")
                    nc.sync.dma_start(out=ssel[:], in_=as_sel[bsl, :])
                    sps = pms.tile([128, 32], F32, tag="pmisc", name=f"sps{bt}")
                    nc.tensor.matmul(sps[:], lhsT=ssel[:], rhs=sgab[:])
                    nc.vector.tensor_tensor(
                        out=M[:, :D], in0=M[:, :D], in1=sps[:],
                        op=mybir.AluOpType.add,
                    )

                # ---- cate seq-sum: count-matmul ----
                cps = pms.tile([128, 32], F32, tag="pmisc", name=f"cps{bt}")
                for cg in range(5 if "c" in PARTS else 0):
                    c0, c1 = cg * 16, min((cg + 1) * 16, 79)
                    r0 = (bt * 79 + c0) * 128
                    cs = cntp.tile([128, 16 * 128], BF16, tag="cnt")
                    nc.sync.dma_start(
                        out=cs[:, : (c1 - c0) * 128].rearrange(
                            "p (c b) -> p c b", c=c1 - c0
                        ),
                        in_=counts[r0 : r0 + (c1 - c0) * 128, :]
                        .rearrange("(c p) b -> c p b", p=128)
                        .transpose([1, 0, 2]),
                    )
                    for c in range(c0, c1):
                        nc.tensor.matmul(
                            cps[:],
                            lhsT=cs[:, (c - c0) * 128 : (c - c0 + 1) * 128],
                            rhs=t1res[:, c * D : (c + 1) * D],
                            start=(c == 0),
                            stop=(c == 78),
                        )
                if "c" in PARTS:
                    nc.vector.tensor_copy(out=M[:, D:], in_=cps[:])
                else:
                    nc.vector.memset(M[:, D:], 0.0)
                nc.vector.tensor_scalar_mul(M[:], M[:], invs[:, bt : bt + 1])

                Pp = sb.tile([128, 2 * D], F32, tag="Pp")
                nc.vector.tensor_tensor(
                    out=Pp[:], in0=I[:], in1=M[:], op=mybir.AluOpType.mult
                )

                # ---- GNN ----
                gacc = pga.tile([128, 128], F32, tag="gacc", name=f"gacc{bt}")
                k = 0
                for sh in range(NSH_G if "g" in PARTS else 0):
                    r0 = (bt * NSH_G + sh) * 128
                    it = sb.tile([128, LG // 16], I16, tag="gidx")
                    nc.sync.dma_start(out=it[:], in_=gidx[r0 : r0 + 128, :])
                    dest = ggp.tile([128, TG * 128], BF16, tag="gdest")
                    nc.gpsimd.dma_gather(
                        out_ap=dest[:].rearrange("p (s e) -> p s e", e=128),
                        in_ap=mem01b[sh * SHG : (sh + 1) * SHG, :],
                        idxs_ap=it[:],
                        num_idxs=LG,
                        num_idxs_reg=LG,
                        elem_size=128,
                        single_packet=False,
                        queue_num=q(),
                    )
                    sl = selp.tile([128, TG * 128], BF16, tag="gsel")
                    nc.sync.dma_start(out=sl[:], in_=gsel[r0 : r0 + 128, :])
                    for j in range(TG):
                        gnn_tile(
                            nc, dest[:, j * 128 : (j + 1) * 128],
                            sl[:, j * 128 : (j + 1) * 128],
                            gacc, identb, wst, bst, xp, pch, k,
                            start=(k == 0), stop=False,
                        )
                        k += 1
                # GNN spill
                if "g" in PARTS:
                    git = sb.tile([128, 1], I32, tag="gspidx")
                    nc.sync.dma_start(out=git[:], in_=gs_idx[bsl, None])
                    gsp = sb.tile([128, 128], BF16, tag="gsp")
                    nc.gpsimd.indirect_dma_start(
                        out=gsp[:], out_offset=None, in_=mem01b[:],
                        in_offset=bass.IndirectOffsetOnAxis(ap=git[:], axis=0),
                    )
                    gssl = sb.tile([128, 128], BF16, tag="gssl")
                    nc.sync.dma_start(out=gssl[:], in_=gs_sel[bsl, :])
                    gnn_tile(nc, gsp[:], gssl[:], gacc, identb, wst, bst, xp, pch,
                             k, start=False, stop=True)

                gnn = sb.tile([128, G], F32, tag="gnn")
                if "g" in PARTS:
                    nc.vector.tensor_copy(out=gnn[:], in_=gacc[:, :G])
                    nc.vector.tensor_tensor(
                        out=gnn[:], in0=gnn[:], in1=gacc[:, G:],
                        op=mybir.AluOpType.add,
                    )
                else:
                    nc.vector.memset(gnn[:], 0.0)

                if DEBUG:
                    for nm, tl in (("dbgU", U), ("dbgI", I), ("dbgM", M), ("dbgG", gnn)):
                        nc.sync.dma_start(out=dbg[nm][bsl, :], in_=tl[:])

                # ---- transpose feature pieces into fT slabs ----
                for pi, piece in enumerate((U, I, M, Pp, gnn)):
                    p_ps = pms.tile([64, 128], F32, tag="pmisc", name=f"pt{bt}_{pi}")
                    nc.tensor.transpose(out=p_ps[:], in_=piece[:], identity=identf[:])
                    slab, row = divmod(pi * 64, 128)
                    nc.vector.tensor_copy(out=fT[slab][row : row + 64, bsl], in_=p_ps[:])

            # ---- MLP ----
            h1s = []
            for m in range(2):
                h1_ps = pms.tile([128, BC], F32, tag="pmisc", name=f"h1ps{m}")
                for kk in range(3):
                    kp = 128 if kk < 2 else 64
                    nc.tensor.matmul(
                        h1_ps[:],
                        lhsT=w1ts[kk][:kp, m * 128 : (m + 1) * 128],
                        rhs=fT[kk][:],
                        start=(kk == 0),
                        stop=(kk == 2),
                    )
                h1 = xp.tile([128, BC], F32, tag="h1", name=f"h1_{m}")
                nc.scalar.activation(
                    out=h1[:], in_=h1_ps[:],
                    func=mybir.ActivationFunctionType.Relu, bias=b1s[m][:, 0:1],
                )
                h1s.append(h1)
            h2_ps = pms.tile([128, BC], F32, tag="pmisc", name="h2ps")
            for m in range(2):
                nc.tensor.matmul(
                    h2_ps[:], lhsT=w2ts[m][:], rhs=h1s[m][:],
                    start=(m == 0), stop=(m == 1),
                )
            h2 = xp.tile([128, BC], F32, tag="h2", name="h2")
            nc.scalar.activation(
                out=h2[:], in_=h2_ps[:],
                func=mybir.ActivationFunctionType.Relu, bias=b2s[:, 0:1],
            )
            lg_ps = pms.tile([1, BC], F32, tag="pmisc", name="lgps")
            nc.tensor.matmul(lg_ps[:], lhsT=w3ts[:], rhs=h2[:])
            lgt = xp.tile([1, BC], F32, tag="lg", name="lgt")
            nc.vector.tensor_scalar_add(lgt[:], lg_ps[:], b3s[:, 0:1])
            nc.sync.dma_start(out=out[None, :], in_=lgt[:])

    nc.compile()
    return nc


def _prep(inp):
    """Host-side input transforms -> per-core in_maps."""
    f32 = np.float32
    bf16 = mybir.dt.np(BF16)
    g = lambda k: np.asarray(inp[k])

    it0 = g("item_tab0").astype(f32)  # [100000, 32]
    it1 = g("item_tab1").astype(f32)  # [10000, 32]
    # bf16 adgroup table, rows padded to 128 elems, sharded into 4x32768 rows
    tab0b = np.zeros((NSH_A * SHA, 128), bf16)
    tab0b[:V0 - 1, :D] = it0.astype(bf16)
    tab0f = np.vstack([it0, np.zeros((V0 - it0.shape[0], D), f32)])
    tab1f = np.vstack([it1, np.zeros((V1 - it1.shape[0], D), f32)])
    tab1b = np.zeros((V1P, D), bf16)
    tab1b[:it1.shape[0]] = it1.astype(bf16)
    mem01 = np.concatenate([g("mem0"), g("mem1")], axis=1).astype(f32)  # [200000,128]
    mem01b = np.zeros((NSH_G * SHG, 128), bf16)
    mem01b[:VM] = mem01.astype(bf16)
    ut0_ = np.ascontiguousarray(g("user_tab0").astype(f32))
    ut1_ = np.ascontiguousarray(g("user_tab1").astype(f32))

    wstack = np.zeros((128, 128), bf16)
    wstack[:G, :G] = g("W_agg0").T.astype(bf16)
    wstack[G:, G:] = g("W_agg1").T.astype(bf16)
    bstack = np.concatenate([g("b_agg0"), g("b_agg1")]).astype(f32)
    w1t = np.ascontiguousarray(g("W1").T.astype(f32))
    w2t = np.ascontiguousarray(g("W2").T.astype(f32))
    w3t = np.ascontiguousarray(g("W3").T.astype(f32))
    b1 = g("b1").astype(f32); b2 = g("b2").astype(f32); b3 = g("b3").astype(f32)

    aseq = g("adgroup_id_seq").astype(np.int64)
    cseq = g("cate_id_seq").astype(np.int64)
    nbr = g("neighbor_ids").astype(np.int64)
    seq_mask = aseq != 0
    invseq_all = (1.0 / np.maximum(seq_mask.sum(-1), 1)).astype(f32)
    nmask = nbr != 0
    invn = (0.5 / np.maximum(nmask.sum(-1), 1)).astype(f32)

    def pack16(stream):
        # [L] -> [128, L//16]: idx k at [k%16, k//16], replicated x8
        w = stream.reshape(-1, 16).T.astype(np.int16)
        return np.tile(w, (8, 1))

    in_maps = []
    for c in range(NC):
        bs = slice(c * BC, (c + 1) * BC)
        a_c, c_c, m_c = aseq[bs], cseq[bs], seq_mask[bs]
        n_c, nm_c = nbr[bs], nmask[bs]
        invn_c = invn[bs]

        aidx_l = np.zeros((NBT * NSH_A * 128, LA // 16), np.int16)
        asel_l = np.zeros((NBT * NSH_A * 128, TA * 32), bf16)
        gidx_l = np.zeros((NBT * NSH_G * 128, LG // 16), np.int16)
        gsel_l = np.zeros((NBT * NSH_G * 128, TG * 128), bf16)
        as_idx_l = np.zeros(NBT * 128, np.int32)
        as_sel_l = np.zeros((NBT * 128, 128), bf16)
        gs_idx_l = np.zeros(NBT * 128, np.int32)
        gs_sel_l = np.zeros((NBT * 128, 128), bf16)
        counts_l = np.zeros((NBT * 79 * 128, 128), bf16)

        for bt in range(NBT):
            btsl = slice(bt * 128, (bt + 1) * 128)
            a = a_c[btsl]; cc = c_c[btsl]; mm = m_c[btsl]
            # flattened (b, s) order, masked
            b_loc = np.repeat(np.arange(128), S)
            av = a.ravel(); mv = mm.ravel()
            b_m = b_loc[mv]; a_m = av[mv]
            sh_a = a_m // SHA; loc_a = a_m % SHA
            spill = []  # (b_loc, global_idx)
            for sh in range(NSH_A):
                pick = sh_a == sh
                bb, ll, gg_ = b_m[pick], loc_a[pick], a_m[pick]
                stream = np.zeros(LA, np.int64)
                selpos, selcol = [], []
                for w in range(NW):
                    inw = (bb // 32) == w
                    lw, bw, gw = ll[inw], bb[inw], gg_[inw]
                    if len(lw) > LW:
                        for z in range(LW, len(lw)):
                            spill.append((bw[z], gw[z]))
                        lw, bw = lw[:LW], bw[:LW]
                    base = w * LW
                    stream[base : base + len(lw)] = lw
                    selpos.append(base + np.arange(len(lw)))
                    selcol.append(bw - 32 * w)
                r0 = (bt * NSH_A + sh) * 128
                aidx_l[r0 : r0 + 128] = pack16(stream)
                sp = np.concatenate(selpos); sc = np.concatenate(selcol)
                tj, pp = sp // 128, sp % 128
                sel = np.zeros((128, TA * 32), f32)
                sel[pp, tj * 32 + sc] = 1.0
                asel_l[r0 : r0 + 128] = sel.astype(bf16)
            assert len(spill) <= 128, f"adgroup spill overflow {len(spill)}"
            for z, (bb, gg_) in enumerate(spill):
                as_idx_l[bt * 128 + z] = gg_
                as_sel_l[bt * 128 + z, bb] = 1.0

            # cate counts [79*128, 128]
            cm = cc.ravel()[mv]
            C = np.bincount(cm * 128 + b_m, minlength=V1P * 128).reshape(V1P, 128)
            counts_l[bt * 79 * 128 : (bt + 1) * 79 * 128] = C.astype(bf16)

            # GNN streams
            nb = n_c[btsl]  # [128, 64]
            msc = (nm_c[btsl].astype(f32) * invn_c[btsl][:, None])  # [128, 64]
            b_loc2 = np.repeat(np.arange(128), N)
            nv = nb.ravel(); mscv = msc.ravel()
            sh_g = nv // SHG; loc_g = nv % SHG
            gspill = []
            for sh in range(NSH_G):
                pick = sh_g == sh
                bb, ll, gg_, ms = b_loc2[pick], loc_g[pick], nv[pick], mscv[pick]
                if len(ll) > LG:
                    for z in range(LG, len(ll)):
                        gspill.append((bb[z], gg_[z], ms[z]))
                    bb, ll, ms = bb[:LG], ll[:LG], ms[:LG]
                stream = np.zeros(LG, np.int64)
                stream[: len(ll)] = ll
                r0 = (bt * NSH_G + sh) * 128
                gidx_l[r0 : r0 + 128] = pack16(stream)
                sp = np.arange(len(ll))
                tj, pp = sp // 128, sp % 128
                sel = np.zeros((128, TG * 128), f32)
                sel[pp, tj * 128 + bb] = ms
                gsel_l[r0 : r0 + 128] = sel.astype(bf16)
            assert len(gspill) <= 128, f"gnn spill overflow {len(gspill)}"
            for z, (bb, gg_, ms) in enumerate(gspill):
                gs_idx_l[bt * 128 + z] = gg_
                gs_sel_l[bt * 128 + z, bb] = ms

        in_maps.append(
            {
                "tab0b": tab0b, "mem01b": mem01b, "tab0f": tab0f, "tab1f": tab1f,
                "ut0": ut0_, "ut1": ut1_, "tab1b": tab1b, "counts": counts_l,
                "iu0": g("user_f0")[bs].astype(np.int32),
                "iu1": g("user_f1")[bs].astype(np.int32),
                "iad": g("adgroup_id")[bs].astype(np.int32),
                "icat": g("cate_id")[bs].astype(np.int32),
                "aidx": aidx_l, "gidx": gidx_l, "asel": asel_l, "gsel": gsel_l,
                "as_idx": as_idx_l, "as_sel": as_sel_l,
                "gs_idx": gs_idx_l, "gs_sel": gs_sel_l,
                "invseq": invseq_all[bs].reshape(NBT, 128).T.copy(),
                "wstack": wstack, "bstack": bstack,
                "w1t": w1t, "b1": b1, "w2t": w2t, "b2": b2, "w3t": w3t, "b3": b3,
            }
        )
    return in_maps


def kernel(**inputs) -> np.ndarray:
    if "nc" not in _CACHE:
        _CACHE["nc"] = _build()
    nc = _CACHE["nc"]
    in_maps = _prep(inputs)
    trace = bool(_os.environ.get("KERNEL_TRACE"))
    res = run_bass_kernel_spmd(nc, in_maps, list(range(NC)), trace=trace)
    _CACHE["last_result"] = res
    out = np.concatenate([res.results[c]["out"] for c in range(NC)])
    return out[:, None].astype(np.float32)



# revision 8
# speedup vs baseline: 1.3419x; 1.3419x over previous
"""Trainium2 Bass kernel for the CIGAR GNN message-passing model (v2).

Data-parallel over batch across 8 NeuronCores (512 rows/core). All bulk
gathers use the custom dma_gather ucode (256B rows, int16 shard-local
indices, 4 SWDGE queues). Host builds compacted (order-preserving) index
streams per (batch-tile, table-shard), padded so segment sums become
selector matmuls with statically-known structure; overflow rows spill to
one per-partition DIRECT2D gather per batch-tile.

  - adgroup seq-sum: bf16 rows padded to 128 elems; 4 shards; streams
    grouped by 32-batch windows (13 tiles each) -> [32,32] PSUM windows.
  - cate seq-sum: count-matmul (exact bf16 integer counts x bf16 table).
  - GNN: bf16 mem0|mem1 256B rows; 7 shards; per-tile PE transpose ->
    blockdiag(W0^T,W1^T) matmul -> tanh -> transpose -> selector matmul
    with mask*0.5/len folded into the bf16 selector values.
  - user/item singles: [P,1] indirect DMA (DIRECT2D), exact f32.
  - MLP on transposed [320, 512] features in f32.
"""

import numpy as np

import concourse.bass as bass
import concourse.bacc as bacc
import concourse.mybir as mybir
import concourse.tile as tile
from concourse.bass_utils import run_bass_kernel_spmd
from concourse.masks import make_identity

NC = 8
B, S, N, D, G = 4096, 200, 64, 32, 64
BC = B // NC  # 512
NBT = BC // 128  # 4
V0, V1, VM = 100001, 10001, 200000
SHA = 25088  # adgroup shard width (int16 range, uniform load)
SHG = 28672  # mem shard width
NSH_A = 4  # adgroup table shards
NSH_G = 7  # mem table shards
LW = 1664  # adgroup rows per (window, shard) block: 13 tiles
TW = LW // 128  # 13
NW = 4  # 32-batch windows per bt
LA = LW * NW  # 6656 rows per (bt, shard) stream
TA = LA // 128  # 52
LG = 1280  # GNN rows per (bt, shard) stream: 10 tiles
TG = LG // 128
V1P = 79 * 128  # padded cate rows (10112)
NQ = 4  # SWDGE queues

F32 = mybir.dt.float32
BF16 = mybir.dt.bfloat16
I16 = mybir.dt.int16
I32 = mybir.dt.int32

DEBUG = False
import os as _os
PARTS = _os.environ.get("KPARTS", "aucgm")  # adgroup,user,cate,gnn,mlp-spills
_CACHE = {}


def _build():
    nc = bacc.Bacc(None, target_bir_lowering=False, num_swdge_queues=NQ)

    # ---- DRAM inputs ----
    tab0b = nc.dram_tensor("tab0b", [NSH_A * SHA, 128], BF16, kind="ExternalInput")
    mem01b = nc.dram_tensor("mem01b", [NSH_G * SHG, 128], BF16, kind="ExternalInput")
    tab0f = nc.dram_tensor("tab0f", [V0, D], F32, kind="ExternalInput")
    tab1f = nc.dram_tensor("tab1f", [V1, D], F32, kind="ExternalInput")
    ut0 = nc.dram_tensor("ut0", [50000, D], F32, kind="ExternalInput")
    ut1 = nc.dram_tensor("ut1", [50000, D], F32, kind="ExternalInput")
    tab1b = nc.dram_tensor("tab1b", [V1P, D], BF16, kind="ExternalInput")
    counts = nc.dram_tensor("counts", [NBT * 79 * 128, 128], BF16, kind="ExternalInput")
    iu0 = nc.dram_tensor("iu0", [BC], I32, kind="ExternalInput")
    iu1 = nc.dram_tensor("iu1", [BC], I32, kind="ExternalInput")
    iad = nc.dram_tensor("iad", [BC], I32, kind="ExternalInput")
    icat = nc.dram_tensor("icat", [BC], I32, kind="ExternalInput")
    aidx = nc.dram_tensor("aidx", [NBT * NSH_A * 128, LA // 16], I16, kind="ExternalInput")
    gidx = nc.dram_tensor("gidx", [NBT * NSH_G * 128, LG // 16], I16, kind="ExternalInput")
    asel = nc.dram_tensor("asel", [NBT * NSH_A * 128, TA * 32], BF16, kind="ExternalInput")
    gsel = nc.dram_tensor("gsel", [NBT * NSH_G * 128, TG * 128], BF16, kind="ExternalInput")
    as_idx = nc.dram_tensor("as_idx", [NBT * 128], I32, kind="ExternalInput")
    as_sel = nc.dram_tensor("as_sel", [NBT * 128, 128], BF16, kind="ExternalInput")
    gs_idx = nc.dram_tensor("gs_idx", [NBT * 128], I32, kind="ExternalInput")
    gs_sel = nc.dram_tensor("gs_sel", [NBT * 128, 128], BF16, kind="ExternalInput")
    invseq = nc.dram_tensor("invseq", [128, NBT], F32, kind="ExternalInput")
    wstack = nc.dram_tensor("wstack", [128, 128], BF16, kind="ExternalInput")
    bstack = nc.dram_tensor("bstack", [128], F32, kind="ExternalInput")
    w1t = nc.dram_tensor("w1t", [320, 256], F32, kind="ExternalInput")
    b1d = nc.dram_tensor("b1", [256], F32, kind="ExternalInput")
    w2t = nc.dram_tensor("w2t", [256, 128], F32, kind="ExternalInput")
    b2d = nc.dram_tensor("b2", [128], F32, kind="ExternalInput")
    w3t = nc.dram_tensor("w3t", [128, 1], F32, kind="ExternalInput")
    b3d = nc.dram_tensor("b3", [1], F32, kind="ExternalInput")
    out = nc.dram_tensor("out", [BC], F32, kind="ExternalOutput")
    warm = nc.dram_tensor("warm", [1, 4], BF16, kind="ExternalOutput")
    dbg = {}
    if DEBUG:
        for nm in ("dbgU", "dbgI", "dbgM", "dbgG"):
            dbg[nm] = nc.dram_tensor(nm, [BC, 64], F32, kind="ExternalOutput")

    qrr = [0]

    def q():  # round-robin SWDGE queue picker
        qrr[0] = (qrr[0] + 1) % NQ
        return qrr[0]

    def gnn_tile(nc, src_ap, sel_ap, gacc, identb, wst, bst, xp, pch, k, start, stop):
        xt_ps = pch.tile([128, 128], BF16, tag="pchain", name=f"x{k % 4}")
        nc.tensor.transpose(out=xt_ps[:], in_=src_ap, identity=identb[:])
        xt = xp.tile([128, 128], BF16, tag="xt", name=f"xt{k % 4}")
        nc.vector.tensor_copy(out=xt[:], in_=xt_ps[:])
        ht_ps = pch.tile([128, 128], F32, tag="pchain", name=f"h{k % 4}")
        nc.tensor.matmul(ht_ps[:], lhsT=wst[:], rhs=xt[:])
        hts = xp.tile([128, 128], BF16, tag="hts", name=f"ht{k % 4}")
        nc.scalar.activation(
            out=hts[:], in_=ht_ps[:],
            func=mybir.ActivationFunctionType.Tanh, bias=bst[:, 0:1],
        )
        h_ps = pch.tile([128, 128], BF16, tag="pchain", name=f"hp{k % 4}")
        nc.tensor.transpose(out=h_ps[:], in_=hts[:], identity=identb[:])
        hh = xp.tile([128, 128], BF16, tag="hh", name=f"hh{k % 4}")
        nc.vector.tensor_copy(out=hh[:], in_=h_ps[:])
        nc.tensor.matmul(gacc[:], lhsT=sel_ap, rhs=hh[:], start=start, stop=stop)

    with tile.TileContext(nc) as tc:
        with (
            tc.tile_pool(name="const", bufs=1) as cpool,
            tc.tile_pool(name="sb", bufs=3) as sb,
            tc.tile_pool(name="idx", bufs=9) as idxp,
            tc.tile_pool(name="ag", bufs=18) as agp,
            tc.tile_pool(name="gg", bufs=10) as ggp,
            tc.tile_pool(name="sel", bufs=6) as selp,
            tc.tile_pool(name="cnt", bufs=3) as cntp,
            tc.tile_pool(name="x", bufs=4) as xp,
            tc.tile_pool(name="pch", bufs=4, space="PSUM") as pch,
            tc.tile_pool(name="pga", bufs=2, space="PSUM") as pga,
            tc.tile_pool(name="pms", bufs=2, space="PSUM") as pms,
        ):
            # ---- constants ----
            identb = cpool.tile([128, 128], BF16)
            make_identity(nc, identb[:])
            identf = cpool.tile([128, 128], F32)
            make_identity(nc, identf[:])
            wst = cpool.tile([128, 128], BF16)
            nc.sync.dma_start(out=wst[:], in_=wstack[:])
            bst = cpool.tile([128, 1], F32)
            nc.sync.dma_start(out=bst[:], in_=bstack[:, None])
            invs = cpool.tile([128, NBT], F32)
            nc.sync.dma_start(out=invs[:], in_=invseq[:])
            w1ts = [cpool.tile([128, 256], F32, tag=f"w1t{k}", name=f"w1t{k}") for k in range(3)]
            for k in range(3):
                lo, hi = k * 128, min((k + 1) * 128, 320)
                nc.sync.dma_start(out=w1ts[k][: hi - lo, :], in_=w1t[lo:hi, :])
            w2ts = [cpool.tile([128, 128], F32, tag=f"w2t{k}", name=f"w2t{k}") for k in range(2)]
            for k in range(2):
                nc.sync.dma_start(out=w2ts[k][:], in_=w2t[k * 128 : (k + 1) * 128, :])
            w3ts = cpool.tile([128, 1], F32)
            nc.sync.dma_start(out=w3ts[:], in_=w3t[:])
            b1s = [cpool.tile([128, 1], F32, tag=f"b1{k}", name=f"b1{k}") for k in range(2)]
            for k in range(2):
                nc.sync.dma_start(out=b1s[k][:], in_=b1d[k * 128 : (k + 1) * 128, None])
            b2s = cpool.tile([128, 1], F32)
            nc.sync.dma_start(out=b2s[:], in_=b2d[:, None])
            b3s = cpool.tile([1, 1], F32)
            nc.sync.dma_start(out=b3s[:], in_=b3d[:, None])
            t1res = cpool.tile([128, 79 * D], BF16)
            if "c" in PARTS:
                nc.sync.dma_start(
                    out=t1res[:].rearrange("p (c f) -> p c f", c=79),
                    in_=tab1b[:]
                    .rearrange("(c p) f -> c p f", p=128)
                    .transpose([1, 0, 2]),
                )
            fT = [
                cpool.tile([128, BC], F32, tag="fT0", name="fT0"),
                cpool.tile([128, BC], F32, tag="fT1", name="fT1"),
                cpool.tile([64, BC], F32, tag="fT2", name="fT2"),
            ]

            # warmup gather: absorb the cold-start synchronous first launch
            if "a" in PARTS or "g" in PARTS:
                wit = cpool.tile([128, 8], I16, tag="warmidx", name="warmidx")
                nc.sync.dma_start(out=wit[:], in_=aidx[0:128, 0:8])
                wdest = cpool.tile([128, 128], BF16, tag="warmdest", name="warmdest")
                nc.gpsimd.dma_gather(
                    out_ap=wdest[:].rearrange("p (s e) -> p s e", e=128),
                    in_ap=tab0b[0:SHA, :],
                    idxs_ap=wit[:],
                    num_idxs=128,
                    num_idxs_reg=128,
                    elem_size=128,
                    single_packet=False,
                    queue_num=0,
                )
                nc.sync.dma_start(out=warm[:, :], in_=wdest[0:1, 0:4])

            for bt in range(NBT):
                bsl = slice(bt * 128, (bt + 1) * 128)

                # ---- adgroup gathers: per-window calls, rr queues ----
                adests, asels, aits = {}, [], []
                for sh in range(NSH_A if "a" in PARTS else 0):
                    r0 = (bt * NSH_A + sh) * 128
                    it = idxp.tile([128, LA // 16], I16, tag="aidx")
                    nc.sync.dma_start(out=it[:], in_=aidx[r0 : r0 + 128, :])
                    aits.append(it)
                    sl = selp.tile([128, TA * 32], BF16, tag="asel")
                    nc.sync.dma_start(out=sl[:], in_=asel[r0 : r0 + 128, :])
                    asels.append(sl)
                for w in range(NW if "a" in PARTS else 0):
                    for sh in range(NSH_A):
                        dest = agp.tile([128, TW * 128], BF16, tag="adest")
                        nc.gpsimd.dma_gather(
                            out_ap=dest[:].rearrange("p (s e) -> p s e", e=128),
                            in_ap=tab0b[sh * SHA : (sh + 1) * SHA, :],
                            idxs_ap=aits[sh][:, w * (LW // 16) : (w + 1) * (LW // 16)],
                            num_idxs=LW,
                            num_idxs_reg=LW,
                            elem_size=128,
                            single_packet=False,
                            queue_num=q(),
                        )
                        adests[(w, sh)] = dest

                # ---- GNN gathers (launch before consumers) ----
                gdests, gsels = [], []
                for sh in range(NSH_G if "g" in PARTS else 0):
                    r0 = (bt * NSH_G + sh) * 128
                    it = idxp.tile([128, LG // 16], I16, tag="gidx")
                    nc.sync.dma_start(out=it[:], in_=gidx[r0 : r0 + 128, :])
                    dest = ggp.tile([128, TG * 128], BF16, tag="gdest")
                    nc.gpsimd.dma_gather(
                        out_ap=dest[:].rearrange("p (s e) -> p s e", e=128),
                        in_ap=mem01b[sh * SHG : (sh + 1) * SHG, :],
                        idxs_ap=it[:],
                        num_idxs=LG,
                        num_idxs_reg=LG,
                        elem_size=128,
                        single_packet=False,
                        queue_num=q(),
                    )
                    gdests.append(dest)
                    sl = selp.tile([128, TG * 128], BF16, tag="gsel")
                    nc.sync.dma_start(out=sl[:], in_=gsel[r0 : r0 + 128, :])
                    gsels.append(sl)

                # ---- user / item embeddings (exact f32 DIRECT2D) ----
                U = sb.tile([128, 2 * D], F32, tag="U")
                I = sb.tile([128, 2 * D], F32, tag="I")
                if "u" not in PARTS:
                    nc.vector.memset(U[:], 0.0)
                    nc.vector.memset(I[:], 0.0)
                for dst, col, idxd, tabd in (
                    (U, 0, iu0, ut0),
                    (U, D, iu1, ut1),
                    (I, 0, iad, tab0f),
                    (I, D, icat, tab1f),
                ) if "u" in PARTS else ():
                    it = sb.tile([128, 1], I32, tag="smallidx")
                    nc.sync.dma_start(out=it[:], in_=idxd[bsl, None])
                    nc.gpsimd.indirect_dma_start(
                        out=dst[:, col : col + D],
                        out_offset=None,
                        in_=tabd[:],
                        in_offset=bass.IndirectOffsetOnAxis(ap=it[:], axis=0),
                    )

                # adgroup spill gather (indirect, after bulk launches)
                if "a" in PARTS:
                    sit = sb.tile([128, 1], I32, tag="spidx")
                    nc.sync.dma_start(out=sit[:], in_=as_idx[bsl, None])
                    sga = sb.tile([128, D], F32, tag="sga")
                    nc.gpsimd.indirect_dma_start(
                        out=sga[:], out_offset=None, in_=tab0f[:],
                        in_offset=bass.IndirectOffsetOnAxis(ap=sit[:], axis=0),
                    )
                # GNN spill gather
                if "g" in PARTS:
                    git = sb.tile([128, 1], I32, tag="gspidx")
                    nc.sync.dma_start(out=git[:], in_=gs_idx[bsl, None])
                    gsp = sb.tile([128, 128], BF16, tag="gsp")
                    nc.gpsimd.indirect_dma_start(
                        out=gsp[:], out_offset=None, in_=mem01b[:],
                        in_offset=bass.IndirectOffsetOnAxis(ap=git[:], axis=0),
                    )

                # ---- adgroup window matmuls ----
                M = sb.tile([128, 2 * D], F32, tag="M")
                if "a" not in PARTS:
                    nc.vector.memset(M[:], 0.0)
                for w in range(NW if "a" in PARTS else 0):
                    wps = pms.tile([32, 32], F32, tag="pmisc", name=f"wps{bt}_{w}")
                    k = 0
                    for sh in range(NSH_A):
                        for j in range(TW):
                            nc.tensor.matmul(
                                wps[:],
                                lhsT=asels[sh][:, (w * TW + j) * 32 : (w * TW + j + 1) * 32],
                                rhs=adests[(w, sh)][:, j * 128 : j * 128 + 32],
                                start=(k == 0),
                                stop=(k == NSH_A * TW - 1),
                            )
                            k += 1
                    nc.vector.tensor_copy(out=M[32 * w : 32 * w + 32, :D], in_=wps[:])
                # adgroup spill matmul
                if "a" in PARTS:
                    sgab = sb.tile([128, D], BF16, tag="sgab")
                    nc.vector.tensor_copy(out=sgab[:], in_=sga[:])
                    ssel = sb.tile([128, 128], BF16, tag="ssel")
                    nc.sync.dma_start(out=ssel[:], in_=as_sel[bsl, :])
                    sps = pms.tile([128, 32], F32, tag="pmisc", name=f"sps{bt}")
                    nc.tensor.matmul(sps[:], lhsT=ssel[:], rhs=sgab[:])
                    nc.vector.tensor_tensor(
                        out=M[:, :D], in0=M[:, :D], in1=sps[:],
                        op=mybir.AluOpType.add,
                    )

                # ---- cate seq-sum: count-matmul ----
                cps = pms.tile([128, 32], F32, tag="pmisc", name=f"cps{bt}")
                for cg in range(5 if "c" in PARTS else 0):
                    c0, c1 = cg * 16, min((cg + 1) * 16, 79)
                    r0 = (bt * 79 + c0) * 128
                    cs = cntp.tile([128, 16 * 128], BF16, tag="cnt")
                    nc.sync.dma_start(
                        out=cs[:, : (c1 - c0) * 128].rearrange(
                            "p (c b) -> p c b", c=c1 - c0
                        ),
                        in_=counts[r0 : r0 + (c1 - c0) * 128, :]
                        .rearrange("(c p) b -> c p b", p=128)
                        .transpose([1, 0, 2]),
                    )
                    for c in range(c0, c1):
                        nc.tensor.matmul(
                            cps[:],
                            lhsT=cs[:, (c - c0) * 128 : (c - c0 + 1) * 128],
                            rhs=t1res[:, c * D : (c + 1) * D],
                            start=(c == 0),
                            stop=(c == 78),
                        )
                if "c" in PARTS:
                    nc.vector.tensor_copy(out=M[:, D:], in_=cps[:])
                else:
                    nc.vector.memset(M[:, D:], 0.0)
                nc.vector.tensor_scalar_mul(M[:], M[:], invs[:, bt : bt + 1])

                Pp = sb.tile([128, 2 * D], F32, tag="Pp")
                nc.vector.tensor_tensor(
                    out=Pp[:], in0=I[:], in1=M[:], op=mybir.AluOpType.mult
                )

                # ---- GNN transform + aggregate (consumes gdests) ----
                gacc = pga.tile([128, 128], F32, tag="gacc", name=f"gacc{bt}")
                k = 0
                for sh in range(NSH_G if "g" in PARTS else 0):
                    dest, sl = gdests[sh], gsels[sh]
                    for j in range(TG):
                        gnn_tile(
                            nc, dest[:, j * 128 : (j + 1) * 128],
                            sl[:, j * 128 : (j + 1) * 128],
                            gacc, identb, wst, bst, xp, pch, k,
                            start=(k == 0), stop=False,
                        )
                        k += 1
                # GNN spill transform
                if "g" in PARTS:
                    gssl = sb.tile([128, 128], BF16, tag="gssl")
                    nc.sync.dma_start(out=gssl[:], in_=gs_sel[bsl, :])
                    gnn_tile(nc, gsp[:], gssl[:], gacc, identb, wst, bst, xp, pch,
                             k, start=False, stop=True)

                gnn = sb.tile([128, G], F32, tag="gnn")
                if "g" in PARTS:
                    nc.vector.tensor_copy(out=gnn[:], in_=gacc[:, :G])
                    nc.vector.tensor_tensor(
                        out=gnn[:], in0=gnn[:], in1=gacc[:, G:],
                        op=mybir.AluOpType.add,
                    )
                else:
                    nc.vector.memset(gnn[:], 0.0)

                if DEBUG:
                    for nm, tl in (("dbgU", U), ("dbgI", I), ("dbgM", M), ("dbgG", gnn)):
                        nc.sync.dma_start(out=dbg[nm][bsl, :], in_=tl[:])

                # ---- transpose feature pieces into fT slabs ----
                for pi, piece in enumerate((U, I, M, Pp, gnn)):
                    p_ps = pms.tile([64, 128], F32, tag="pmisc", name=f"pt{bt}_{pi}")
                    nc.tensor.transpose(out=p_ps[:], in_=piece[:], identity=identf[:])
                    slab, row = divmod(pi * 64, 128)
                    nc.vector.tensor_copy(out=fT[slab][row : row + 64, bsl], in_=p_ps[:])

            # ---- MLP ----
            h1s = []
            for m in range(2):
                h1_ps = pms.tile([128, BC], F32, tag="pmisc", name=f"h1ps{m}")
                for kk in range(3):
                    kp = 128 if kk < 2 else 64
                    nc.tensor.matmul(
                        h1_ps[:],
                        lhsT=w1ts[kk][:kp, m * 128 : (m + 1) * 128],
                        rhs=fT[kk][:],
                        start=(kk == 0),
                        stop=(kk == 2),
                    )
                h1 = xp.tile([128, BC], F32, tag="h1", name=f"h1_{m}")
                nc.scalar.activation(
                    out=h1[:], in_=h1_ps[:],
                    func=mybir.ActivationFunctionType.Relu, bias=b1s[m][:, 0:1],
                )
                h1s.append(h1)
            h2_ps = pms.tile([128, BC], F32, tag="pmisc", name="h2ps")
            for m in range(2):
                nc.tensor.matmul(
                    h2_ps[:], lhsT=w2ts[m][:], rhs=h1s[m][:],
                    start=(m == 0), stop=(m == 1),
                )
            h2 = xp.tile([128, BC], F32, tag="h2", name="h2")
            nc.scalar.activation(
                out=h2[:], in_=h2_ps[:],
                func=mybir.ActivationFunctionType.Relu, bias=b2s[:, 0:1],
            )
            lg_ps = pms.tile([1, BC], F32, tag="pmisc", name="lgps")
            nc.tensor.matmul(lg_ps[:], lhsT=w3ts[:], rhs=h2[:])
            lgt = xp.tile([1, BC], F32, tag="lg", name="lgt")
            nc.vector.tensor_scalar_add(lgt[:], lg_ps[:], b3s[:, 0:1])
            nc.sync.dma_start(out=out[None, :], in_=lgt[:])

    nc.compile()
    return nc


def _prep(inp):
    """Host-side input transforms -> per-core in_maps."""
    f32 = np.float32
    bf16 = mybir.dt.np(BF16)
    g = lambda k: np.asarray(inp[k])

    it0 = g("item_tab0").astype(f32)  # [100000, 32]
    it1 = g("item_tab1").astype(f32)  # [10000, 32]
    # bf16 adgroup table, rows padded to 128 elems, sharded into 4x32768 rows
    tab0b = np.zeros((NSH_A * SHA, 128), bf16)
    tab0b[:V0 - 1, :D] = it0.astype(bf16)
    tab0f = np.vstack([it0, np.zeros((V0 - it0.shape[0], D), f32)])
    tab1f = np.vstack([it1, np.zeros((V1 - it1.shape[0], D), f32)])
    tab1b = np.zeros((V1P, D), bf16)
    tab1b[:it1.shape[0]] = it1.astype(bf16)
    mem01 = np.concatenate([g("mem0"), g("mem1")], axis=1).astype(f32)  # [200000,128]
    mem01b = np.zeros((NSH_G * SHG, 128), bf16)
    mem01b[:VM] = mem01.astype(bf16)
    ut0_ = np.ascontiguousarray(g("user_tab0").astype(f32))
    ut1_ = np.ascontiguousarray(g("user_tab1").astype(f32))

    wstack = np.zeros((128, 128), bf16)
    wstack[:G, :G] = g("W_agg0").T.astype(bf16)
    wstack[G:, G:] = g("W_agg1").T.astype(bf16)
    bstack = np.concatenate([g("b_agg0"), g("b_agg1")]).astype(f32)
    w1t = np.ascontiguousarray(g("W1").T.astype(f32))
    w2t = np.ascontiguousarray(g("W2").T.astype(f32))
    w3t = np.ascontiguousarray(g("W3").T.astype(f32))
    b1 = g("b1").astype(f32); b2 = g("b2").astype(f32); b3 = g("b3").astype(f32)

    aseq = g("adgroup_id_seq").astype(np.int64)
    cseq = g("cate_id_seq").astype(np.int64)
    nbr = g("neighbor_ids").astype(np.int64)
    seq_mask = aseq != 0
    invseq_all = (1.0 / np.maximum(seq_mask.sum(-1), 1)).astype(f32)
    nmask = nbr != 0
    invn = (0.5 / np.maximum(nmask.sum(-1), 1)).astype(f32)

    def pack16(stream):
        # [L] -> [128, L//16]: idx k at [k%16, k//16], replicated x8
        w = stream.reshape(-1, 16).T.astype(np.int16)
        return np.tile(w, (8, 1))

    in_maps = []
    for c in range(NC):
        bs = slice(c * BC, (c + 1) * BC)
        a_c, c_c, m_c = aseq[bs], cseq[bs], seq_mask[bs]
        n_c, nm_c = nbr[bs], nmask[bs]
        invn_c = invn[bs]

        aidx_l = np.zeros((NBT * NSH_A * 128, LA // 16), np.int16)
        asel_l = np.zeros((NBT * NSH_A * 128, TA * 32), bf16)
        gidx_l = np.zeros((NBT * NSH_G * 128, LG // 16), np.int16)
        gsel_l = np.zeros((NBT * NSH_G * 128, TG * 128), bf16)
        as_idx_l = np.zeros(NBT * 128, np.int32)
        as_sel_l = np.zeros((NBT * 128, 128), bf16)
        gs_idx_l = np.zeros(NBT * 128, np.int32)
        gs_sel_l = np.zeros((NBT * 128, 128), bf16)
        counts_l = np.zeros((NBT * 79 * 128, 128), bf16)

        for bt in range(NBT):
            btsl = slice(bt * 128, (bt + 1) * 128)
            a = a_c[btsl]; cc = c_c[btsl]; mm = m_c[btsl]
            # flattened (b, s) order, masked
            b_loc = np.repeat(np.arange(128), S)
            av = a.ravel(); mv = mm.ravel()
            b_m = b_loc[mv]; a_m = av[mv]
            sh_a = a_m // SHA; loc_a = a_m % SHA
            spill = []  # (b_loc, global_idx)
            for sh in range(NSH_A):
                pick = sh_a == sh
                bb, ll, gg_ = b_m[pick], loc_a[pick], a_m[pick]
                stream = np.zeros(LA, np.int64)
                selpos, selcol = [], []
                for w in range(NW):
                    inw = (bb // 32) == w
                    lw, bw, gw = ll[inw], bb[inw], gg_[inw]
                    if len(lw) > LW:
                        for z in range(LW, len(lw)):
                            spill.append((bw[z], gw[z]))
                        lw, bw = lw[:LW], bw[:LW]
                    base = w * LW
                    stream[base : base + len(lw)] = lw
                    selpos.append(base + np.arange(len(lw)))
                    selcol.append(bw - 32 * w)
                r0 = (bt * NSH_A + sh) * 128
                aidx_l[r0 : r0 + 128] = pack16(stream)
                sp = np.concatenate(selpos); sc = np.concatenate(selcol)
                tj, pp = sp // 128, sp % 128
                sel = np.zeros((128, TA * 32), f32)
                sel[pp, tj * 32 + sc] = 1.0
                asel_l[r0 : r0 + 128] = sel.astype(bf16)
            assert len(spill) <= 128, f"adgroup spill overflow {len(spill)}"
            for z, (bb, gg_) in enumerate(spill):
                as_idx_l[bt * 128 + z] = gg_
                as_sel_l[bt * 128 + z, bb] = 1.0

            # cate counts [79*128, 128]
            cm = cc.ravel()[mv]
            C = np.bincount(cm * 128 + b_m, minlength=V1P * 128).reshape(V1P, 128)
            counts_l[bt * 79 * 128 : (bt + 1) * 79 * 128] = C.astype(bf16)

            # GNN streams
            nb = n_c[btsl]  # [128, 64]
            msc = (nm_c[btsl].astype(f32) * invn_c[btsl][:, None])  # [128, 64]
            b_loc2 = np.repeat(np.arange(128), N)
            nv = nb.ravel(); mscv = msc.ravel()
            sh_g = nv // SHG; loc_g = nv % SHG
            gspill = []
            for sh in range(NSH_G):
                pick = sh_g == sh
                bb, ll, gg_, ms = b_loc2[pick], loc_g[pick], nv[pick], mscv[pick]
                if len(ll) > LG:
                    for z in range(LG, len(ll)):
                        gspill.append((bb[z], gg_[z], ms[z]))
                    bb, ll, ms = bb[:LG], ll[:LG], ms[:LG]
                stream = np.zeros(LG, np.int64)
                stream[: len(ll)] = ll
                r0 = (bt * NSH_G + sh) * 128
                gidx_l[r0 : r0 + 128] = pack16(stream)
                sp = np.arange(len(ll))
                tj, pp = sp // 128, sp % 128
                sel = np.zeros((128, TG * 128), f32)
                sel[pp, tj * 128 + bb] = ms
                gsel_l[r0 : r0 + 128] = sel.astype(bf16)
            assert len(gspill) <= 128, f"gnn spill overflow {len(gspill)}"
            for z, (bb, gg_, ms) in enumerate(gspill):
                gs_idx_l[bt * 128 + z] = gg_
                gs_sel_l[bt * 128 + z, bb] = ms

        in_maps.append(
            {
                "tab0b": tab0b, "mem01b": mem01b, "tab0f": tab0f, "tab1f": tab1f,
                "ut0": ut0_, "ut1": ut1_, "tab1b": tab1b, "counts": counts_l,
                "iu0": g("user_f0")[bs].astype(np.int32),
                "iu1": g("user_f1")[bs].astype(np.int32),
                "iad": g("adgroup_id")[bs].astype(np.int32),
                "icat": g("cate_id")[bs].astype(np.int32),
                "aidx": aidx_l, "gidx": gidx_l, "asel": asel_l, "gsel": gsel_l,
                "as_idx": as_idx_l, "as_sel": as_sel_l,
                "gs_idx": gs_idx_l, "gs_sel": gs_sel_l,
                "invseq": invseq_all[bs].reshape(NBT, 128).T.copy(),
                "wstack": wstack, "bstack": bstack,
                "w1t": w1t, "b1": b1, "w2t": w2t, "b2": b2, "w3t": w3t, "b3": b3,
            }
        )
    return in_maps


def kernel(**inputs) -> np.ndarray:
    if "nc" not in _CACHE:
        _CACHE["nc"] = _build()
    nc = _CACHE["nc"]
    in_maps = _prep(inputs)
    trace = bool(_os.environ.get("KERNEL_TRACE"))
    res = run_bass_kernel_spmd(nc, in_maps, list(range(NC)), trace=trace)
    _CACHE["last_result"] = res
    out = np.concatenate([res.results[c]["out"] for c in range(NC)])
    return out[:, None].astype(np.float32)



# revision 17
# speedup vs baseline: 1.7964x; 1.3387x over previous
"""Trainium2 Bass kernel for the CIGAR GNN message-passing model (v2).

Data-parallel over batch across 8 NeuronCores (512 rows/core). All bulk
gathers use the custom dma_gather ucode (256B rows, int16 shard-local
indices, 4 SWDGE queues). Host builds compacted (order-preserving) index
streams per (batch-tile, table-shard), padded so segment sums become
selector matmuls with statically-known structure; overflow rows spill to
one per-partition DIRECT2D gather per batch-tile.

  - adgroup seq-sum: bf16 rows padded to 128 elems; 4 shards; streams
    grouped by 32-batch windows (13 tiles each) -> [32,32] PSUM windows.
  - cate seq-sum: count-matmul (exact bf16 integer counts x bf16 table).
  - GNN: bf16 mem0|mem1 256B rows; 7 shards; per-tile PE transpose ->
    blockdiag(W0^T,W1^T) matmul -> tanh -> transpose -> selector matmul
    with mask*0.5/len folded into the bf16 selector values.
  - user/item singles: [P,1] indirect DMA (DIRECT2D), exact f32.
  - MLP on transposed [320, 512] features in f32.
"""

import numpy as np

import concourse.bass as bass
import concourse.bacc as bacc
import concourse.mybir as mybir
import concourse.tile as tile
from concourse.bass_utils import run_bass_kernel_spmd
from concourse.masks import make_identity

NC = 8
B, S, N, D, G = 4096, 200, 64, 32, 64
BC = B // NC  # 512
NBT = BC // 128  # 4
V0, V1, VM = 100001, 10001, 200000
SHA = 25088  # adgroup shard width (int16 range, uniform load)
SHG = 28672  # mem shard width
NSH_A = 4  # adgroup table shards
NSH_G = 7  # mem table shards
LW = 1664  # adgroup rows per (window, shard) block: 13 tiles
TW = LW // 128  # 13
NW = 4  # 32-batch windows per bt
LA = LW * NW  # 6656 rows per (bt, shard) stream
TA = LA // 128  # 52
LG = 1280  # GNN rows per (bt, shard) stream: 10 tiles
TG = LG // 128
V1P = 79 * 128  # padded cate rows (10112)
NQ = 4  # SWDGE queues

F32 = mybir.dt.float32
BF16 = mybir.dt.bfloat16
I16 = mybir.dt.int16
I32 = mybir.dt.int32

DEBUG = False
import os as _os
PARTS = _os.environ.get("KPARTS", "aucgm")  # adgroup,user,cate,gnn,mlp-spills
_CACHE = {}


def _build():
    nc = bacc.Bacc(None, target_bir_lowering=False, num_swdge_queues=NQ)

    # ---- DRAM inputs ----
    tab0b = nc.dram_tensor("tab0b", [NSH_A * SHA, 128], BF16, kind="ExternalInput")
    mem01b = nc.dram_tensor("mem01b", [NSH_G * SHG, 128], BF16, kind="ExternalInput")
    tab0f = nc.dram_tensor("tab0f", [V0, D], F32, kind="ExternalInput")
    tab1f = nc.dram_tensor("tab1f", [V1, D], F32, kind="ExternalInput")
    ut0 = nc.dram_tensor("ut0", [50000, D], F32, kind="ExternalInput")
    ut1 = nc.dram_tensor("ut1", [50000, D], F32, kind="ExternalInput")
    tab1b = nc.dram_tensor("tab1b", [V1P, D], BF16, kind="ExternalInput")
    counts = nc.dram_tensor("counts", [NBT * 79 * 128, 128], BF16, kind="ExternalInput")
    iu0 = nc.dram_tensor("iu0", [BC], I32, kind="ExternalInput")
    iu1 = nc.dram_tensor("iu1", [BC], I32, kind="ExternalInput")
    iad = nc.dram_tensor("iad", [BC], I32, kind="ExternalInput")
    icat = nc.dram_tensor("icat", [BC], I32, kind="ExternalInput")
    aidx = nc.dram_tensor("aidx", [NBT * NSH_A * 128, LA // 16], I16, kind="ExternalInput")
    gidx = nc.dram_tensor("gidx", [NBT * NSH_G * 128, LG // 16], I16, kind="ExternalInput")
    asel = nc.dram_tensor("asel", [NBT * NSH_A * 128, TA * 32], BF16, kind="ExternalInput")
    gsel = nc.dram_tensor("gsel", [NBT * NSH_G * 128, TG * 128], BF16, kind="ExternalInput")
    as_idx = nc.dram_tensor("as_idx", [NBT * 128], I32, kind="ExternalInput")
    as_sel = nc.dram_tensor("as_sel", [NBT * 128, 128], BF16, kind="ExternalInput")
    gs_idx = nc.dram_tensor("gs_idx", [NBT * 128], I32, kind="ExternalInput")
    gs_sel = nc.dram_tensor("gs_sel", [NBT * 128, 128], BF16, kind="ExternalInput")
    invseq = nc.dram_tensor("invseq", [128, NBT], F32, kind="ExternalInput")
    wstack = nc.dram_tensor("wstack", [128, 128], BF16, kind="ExternalInput")
    bstack = nc.dram_tensor("bstack", [128], F32, kind="ExternalInput")
    w1t = nc.dram_tensor("w1t", [320, 256], F32, kind="ExternalInput")
    b1d = nc.dram_tensor("b1", [256], F32, kind="ExternalInput")
    w2t = nc.dram_tensor("w2t", [256, 128], F32, kind="ExternalInput")
    b2d = nc.dram_tensor("b2", [128], F32, kind="ExternalInput")
    w3t = nc.dram_tensor("w3t", [128, 1], F32, kind="ExternalInput")
    b3d = nc.dram_tensor("b3", [1], F32, kind="ExternalInput")
    out = nc.dram_tensor("out", [BC], F32, kind="ExternalOutput")
    warm = nc.dram_tensor("warm", [1, 4], BF16, kind="ExternalOutput")
    dbg = {}
    if DEBUG:
        for nm in ("dbgU", "dbgI", "dbgM", "dbgG"):
            dbg[nm] = nc.dram_tensor(nm, [BC, 64], F32, kind="ExternalOutput")

    qrr = [0]

    def q():  # round-robin SWDGE queue picker
        qrr[0] = (qrr[0] + 1) % NQ
        return qrr[0]

    def gnn_group(nc, src_ap, sel_ap, gw, kbase, pick, identb, wst, bst, xp, pch):
        """Process gw (<=4) gathered 128-row tiles through one wide chain pass.

        src_ap/sel_ap: [128, gw*128] slices. pick(k) -> (gacc, start, stop).
        """
        gid = kbase % 8
        xt_ps = pch.tile([128, gw * 128], BF16, tag="pchain", name=f"x{gid}")
        for i in range(gw):
            nc.tensor.transpose(
                out=xt_ps[:, i * 128 : (i + 1) * 128],
                in_=src_ap[:, i * 128 : (i + 1) * 128],
                identity=identb[:],
            )
        xt = xp.tile([128, gw * 128], BF16, tag="xt", name=f"xt{gid}")
        nc.vector.tensor_copy(out=xt[:], in_=xt_ps[:])
        ht_ps = pch.tile([128, gw * 128], F32, tag="pchain", name=f"h{gid}")
        nc.tensor.matmul(ht_ps[:], lhsT=wst[:], rhs=xt[:])
        hts = xp.tile([128, gw * 128], BF16, tag="hts", name=f"ht{gid}")
        nc.scalar.activation(
            out=hts[:], in_=ht_ps[:],
            func=mybir.ActivationFunctionType.Tanh, bias=bst[:, 0:1],
        )
        h_ps = pch.tile([128, gw * 128], BF16, tag="pchain", name=f"hp{gid}")
        for i in range(gw):
            nc.tensor.transpose(
                out=h_ps[:, i * 128 : (i + 1) * 128],
                in_=hts[:, i * 128 : (i + 1) * 128],
                identity=identb[:],
            )
        hh = xp.tile([128, gw * 128], BF16, tag="hh", name=f"hh{gid}")
        nc.vector.tensor_copy(out=hh[:], in_=h_ps[:])
        for i in range(gw):
            gacc, start, stop = pick(kbase + i)
            nc.tensor.matmul(
                gacc[:],
                lhsT=sel_ap[:, i * 128 : (i + 1) * 128],
                rhs=hh[:, i * 128 : (i + 1) * 128],
                start=start, stop=stop,
            )

    with tile.TileContext(nc) as tc:
        with (
            tc.tile_pool(name="const", bufs=1) as cpool,
            tc.tile_pool(name="sb", bufs=3) as sb,
            tc.tile_pool(name="idx", bufs=9) as idxp,
            tc.tile_pool(name="ag", bufs=18) as agp,
            tc.tile_pool(name="gg", bufs=14) as ggp,
            tc.tile_pool(name="sel", bufs=6) as selp,
            tc.tile_pool(name="cnt", bufs=3) as cntp,
            tc.tile_pool(name="x", bufs=4) as xp,
            tc.tile_pool(name="mlp", bufs=2) as mlpp,
            tc.tile_pool(name="pch", bufs=4, space="PSUM") as pch,
            tc.tile_pool(name="pga", bufs=2, space="PSUM") as pga,
            tc.tile_pool(name="pms", bufs=2, space="PSUM") as pms,
        ):
            # ---- constants ----
            identb = cpool.tile([128, 128], BF16)
            make_identity(nc, identb[:])
            identf = cpool.tile([128, 128], F32)
            make_identity(nc, identf[:])
            wst = cpool.tile([128, 128], BF16)
            nc.sync.dma_start(out=wst[:], in_=wstack[:])
            bst = cpool.tile([128, 1], F32)
            nc.sync.dma_start(out=bst[:], in_=bstack[:, None])
            invs = cpool.tile([128, NBT], F32)
            nc.sync.dma_start(out=invs[:], in_=invseq[:])
            w1ts = [cpool.tile([128, 256], F32, tag=f"w1t{k}", name=f"w1t{k}") for k in range(3)]
            for k in range(3):
                lo, hi = k * 128, min((k + 1) * 128, 320)
                nc.sync.dma_start(out=w1ts[k][: hi - lo, :], in_=w1t[lo:hi, :])
            w2ts = [cpool.tile([128, 128], F32, tag=f"w2t{k}", name=f"w2t{k}") for k in range(2)]
            for k in range(2):
                nc.sync.dma_start(out=w2ts[k][:], in_=w2t[k * 128 : (k + 1) * 128, :])
            w3ts = cpool.tile([128, 1], F32)
            nc.sync.dma_start(out=w3ts[:], in_=w3t[:])
            b1s = [cpool.tile([128, 1], F32, tag=f"b1{k}", name=f"b1{k}") for k in range(2)]
            for k in range(2):
                nc.sync.dma_start(out=b1s[k][:], in_=b1d[k * 128 : (k + 1) * 128, None])
            b2s = cpool.tile([128, 1], F32)
            nc.sync.dma_start(out=b2s[:], in_=b2d[:, None])
            b3s = cpool.tile([1, 1], F32)
            nc.sync.dma_start(out=b3s[:], in_=b3d[:, None])
            t1res = cpool.tile([128, 79 * D], BF16)
            if "c" in PARTS:
                nc.sync.dma_start(
                    out=t1res[:].rearrange("p (c f) -> p c f", c=79),
                    in_=tab1b[:]
                    .rearrange("(c p) f -> c p f", p=128)
                    .transpose([1, 0, 2]),
                )
            fT = [
                cpool.tile([128, BC], F32, tag="fT0", name="fT0"),
                cpool.tile([128, BC], F32, tag="fT1", name="fT1"),
                cpool.tile([64, BC], F32, tag="fT2", name="fT2"),
            ]

            # warmup gather: absorb the cold-start synchronous first launch
            if "a" in PARTS or "g" in PARTS:
                wit = cpool.tile([128, 8], I16, tag="warmidx", name="warmidx")
                nc.sync.dma_start(out=wit[:], in_=aidx[0:128, 0:8])
                wdest = cpool.tile([128, 128], BF16, tag="warmdest", name="warmdest")
                nc.gpsimd.dma_gather(
                    out_ap=wdest[:].rearrange("p (s e) -> p s e", e=128),
                    in_ap=tab0b[0:SHA, :],
                    idxs_ap=wit[:],
                    num_idxs=128,
                    num_idxs_reg=128,
                    elem_size=128,
                    single_packet=False,
                    queue_num=0,
                )
                nc.sync.dma_start(out=warm[:, :], in_=wdest[0:1, 0:4])

            for bt in range(NBT):
                bsl = slice(bt * 128, (bt + 1) * 128)

                # ---- GNN gathers first (their consumer chain is the long pole) ----
                gdests, gsels = [], []
                for sh in range(NSH_G if "g" in PARTS else 0):
                    r0 = (bt * NSH_G + sh) * 128
                    it = idxp.tile([128, LG // 16], I16, tag="gidx")
                    nc.sync.dma_start(out=it[:], in_=gidx[r0 : r0 + 128, :])
                    dest = ggp.tile([128, TG * 128], BF16, tag="gdest")
                    nc.gpsimd.dma_gather(
                        out_ap=dest[:].rearrange("p (s e) -> p s e", e=128),
                        in_ap=mem01b[sh * SHG : (sh + 1) * SHG, :],
                        idxs_ap=it[:],
                        num_idxs=LG,
                        num_idxs_reg=LG,
                        elem_size=128,
                        single_packet=False,
                        queue_num=q(),
                    )
                    gdests.append(dest)
                    sl = selp.tile([128, TG * 128], BF16, tag="gsel")
                    nc.sync.dma_start(out=sl[:], in_=gsel[r0 : r0 + 128, :])
                    gsels.append(sl)

                # ---- adgroup gathers: per-window calls, rr queues ----
                adests, asels, aits = {}, [], []
                for sh in range(NSH_A if "a" in PARTS else 0):
                    r0 = (bt * NSH_A + sh) * 128
                    it = idxp.tile([128, LA // 16], I16, tag="aidx")
                    nc.sync.dma_start(out=it[:], in_=aidx[r0 : r0 + 128, :])
                    aits.append(it)
                    sl = selp.tile([128, TA * 32], BF16, tag="asel")
                    nc.sync.dma_start(out=sl[:], in_=asel[r0 : r0 + 128, :])
                    asels.append(sl)
                for w in range(NW if "a" in PARTS else 0):
                    for sh in range(NSH_A):
                        dest = agp.tile([128, TW * 128], BF16, tag="adest")
                        nc.gpsimd.dma_gather(
                            out_ap=dest[:].rearrange("p (s e) -> p s e", e=128),
                            in_ap=tab0b[sh * SHA : (sh + 1) * SHA, :],
                            idxs_ap=aits[sh][:, w * (LW // 16) : (w + 1) * (LW // 16)],
                            num_idxs=LW,
                            num_idxs_reg=LW,
                            elem_size=128,
                            single_packet=False,
                            queue_num=q(),
                        )
                        adests[(w, sh)] = dest

                # ---- user / item embeddings (exact f32 DIRECT2D) ----
                U = sb.tile([128, 2 * D], F32, tag="U")
                I = sb.tile([128, 2 * D], F32, tag="I")
                if "u" not in PARTS:
                    nc.vector.memset(U[:], 0.0)
                    nc.vector.memset(I[:], 0.0)
                for dst, col, idxd, tabd in (
                    (U, 0, iu0, ut0),
                    (U, D, iu1, ut1),
                    (I, 0, iad, tab0f),
                    (I, D, icat, tab1f),
                ) if "u" in PARTS else ():
                    it = sb.tile([128, 1], I32, tag="smallidx")
                    nc.sync.dma_start(out=it[:], in_=idxd[bsl, None])
                    nc.gpsimd.indirect_dma_start(
                        out=dst[:, col : col + D],
                        out_offset=None,
                        in_=tabd[:],
                        in_offset=bass.IndirectOffsetOnAxis(ap=it[:], axis=0),
                    )

                # adgroup spill gather (indirect, after bulk launches)
                if "a" in PARTS:
                    sit = sb.tile([128, 1], I32, tag="spidx")
                    nc.sync.dma_start(out=sit[:], in_=as_idx[bsl, None])
                    sga = sb.tile([128, D], F32, tag="sga")
                    nc.gpsimd.indirect_dma_start(
                        out=sga[:], out_offset=None, in_=tab0f[:],
                        in_offset=bass.IndirectOffsetOnAxis(ap=sit[:], axis=0),
                    )
                # GNN spill gather
                if "g" in PARTS:
                    git = sb.tile([128, 1], I32, tag="gspidx")
                    nc.sync.dma_start(out=git[:], in_=gs_idx[bsl, None])
                    gsp = sb.tile([128, 128], BF16, tag="gsp")
                    nc.gpsimd.indirect_dma_start(
                        out=gsp[:], out_offset=None, in_=mem01b[:],
                        in_offset=bass.IndirectOffsetOnAxis(ap=git[:], axis=0),
                    )

                # ---- GNN transform + aggregate (4-packed chain, 2 PSUM accums) ----
                nt_g = NSH_G * TG + 1  # 71 tiles incl. spill
                gaccA = pga.tile([128, 128], F32, tag="gacc", name=f"gaccA{bt}")
                gaccB = pga.tile([128, 128], F32, tag="gacc", name=f"gaccB{bt}")
                lastA = ((nt_g - 1) // 2) * 2
                lastB = ((nt_g - 2) // 2) * 2 + 1

                def pick(k):
                    return (
                        gaccA if k % 2 == 0 else gaccB,
                        k < 2,
                        k in (lastA, lastB),
                    )

                k = 0
                for sh in range(NSH_G if "g" in PARTS else 0):
                    dest, sl = gdests[sh], gsels[sh]
                    for j0 in range(0, TG, 4):
                        gw = min(4, TG - j0)
                        gnn_group(
                            nc, dest[:, j0 * 128 : (j0 + gw) * 128],
                            sl[:, j0 * 128 : (j0 + gw) * 128],
                            gw, k, pick, identb, wst, bst, xp, pch,
                        )
                        k += gw
                # GNN spill transform
                if "g" in PARTS:
                    gssl = sb.tile([128, 128], BF16, tag="gssl")
                    nc.sync.dma_start(out=gssl[:], in_=gs_sel[bsl, :])
                    gnn_group(nc, gsp[:], gssl[:], 1, k, pick,
                              identb, wst, bst, xp, pch)

                gnn = sb.tile([128, G], F32, tag="gnn")
                if "g" in PARTS:
                    nc.vector.tensor_copy(out=gnn[:], in_=gaccA[:, :G])
                    nc.vector.tensor_tensor(
                        out=gnn[:], in0=gnn[:], in1=gaccA[:, G:],
                        op=mybir.AluOpType.add,
                    )
                    nc.vector.tensor_tensor(
                        out=gnn[:], in0=gnn[:], in1=gaccB[:, :G],
                        op=mybir.AluOpType.add,
                    )
                    nc.vector.tensor_tensor(
                        out=gnn[:], in0=gnn[:], in1=gaccB[:, G:],
                        op=mybir.AluOpType.add,
                    )
                else:
                    nc.vector.memset(gnn[:], 0.0)

                # ---- adgroup window matmuls ----
                M = sb.tile([128, 2 * D], F32, tag="M")
                if "a" not in PARTS:
                    nc.vector.memset(M[:], 0.0)
                for w in range(NW if "a" in PARTS else 0):
                    wps = pms.tile([32, 32], F32, tag="pmisc", name=f"wps{bt}_{w}")
                    k = 0
                    for sh in range(NSH_A):
                        for j in range(TW):
                            nc.tensor.matmul(
                                wps[:],
                                lhsT=asels[sh][:, (w * TW + j) * 32 : (w * TW + j + 1) * 32],
                                rhs=adests[(w, sh)][:, j * 128 : j * 128 + 32],
                                start=(k == 0),
                                stop=(k == NSH_A * TW - 1),
                            )
                            k += 1
                    nc.vector.tensor_copy(out=M[32 * w : 32 * w + 32, :D], in_=wps[:])
                # adgroup spill matmul
                if "a" in PARTS:
                    sgab = sb.tile([128, D], BF16, tag="sgab")
                    nc.vector.tensor_copy(out=sgab[:], in_=sga[:])
                    ssel = sb.tile([128, 128], BF16, tag="ssel")
                    nc.sync.dma_start(out=ssel[:], in_=as_sel[bsl, :])
                    sps = pms.tile([128, 32], F32, tag="pmisc", name=f"sps{bt}")
                    nc.tensor.matmul(sps[:], lhsT=ssel[:], rhs=sgab[:])
                    nc.vector.tensor_tensor(
                        out=M[:, :D], in0=M[:, :D], in1=sps[:],
                        op=mybir.AluOpType.add,
                    )

                # ---- cate seq-sum: count-matmul ----
                cps = pms.tile([128, 32], F32, tag="pmisc", name=f"cps{bt}")
                for cg in range(5 if "c" in PARTS else 0):
                    c0, c1 = cg * 16, min((cg + 1) * 16, 79)
                    r0 = (bt * 79 + c0) * 128
                    cs = cntp.tile([128, 16 * 128], BF16, tag="cnt")
                    nc.sync.dma_start(
                        out=cs[:, : (c1 - c0) * 128].rearrange(
                            "p (c b) -> p c b", c=c1 - c0
                        ),
                        in_=counts[r0 : r0 + (c1 - c0) * 128, :]
                        .rearrange("(c p) b -> c p b", p=128)
                        .transpose([1, 0, 2]),
                    )
                    for c in range(c0, c1):
                        nc.tensor.matmul(
                            cps[:],
                            lhsT=cs[:, (c - c0) * 128 : (c - c0 + 1) * 128],
                            rhs=t1res[:, c * D : (c + 1) * D],
                            start=(c == 0),
                            stop=(c == 78),
                        )
                if "c" in PARTS:
                    nc.vector.tensor_copy(out=M[:, D:], in_=cps[:])
                else:
                    nc.vector.memset(M[:, D:], 0.0)
                nc.vector.tensor_scalar_mul(M[:], M[:], invs[:, bt : bt + 1])

                Pp = sb.tile([128, 2 * D], F32, tag="Pp")
                nc.vector.tensor_tensor(
                    out=Pp[:], in0=I[:], in1=M[:], op=mybir.AluOpType.mult
                )

                if DEBUG:
                    for nm, tl in (("dbgU", U), ("dbgI", I), ("dbgM", M), ("dbgG", gnn)):
                        nc.sync.dma_start(out=dbg[nm][bsl, :], in_=tl[:])

                # ---- transpose feature pieces into fT slabs ----
                for pi, piece in enumerate((U, I, M, Pp, gnn)):
                    p_ps = pms.tile([64, 128], F32, tag="pmisc", name=f"pt{bt}_{pi}")
                    nc.tensor.transpose(out=p_ps[:], in_=piece[:], identity=identf[:])
                    slab, row = divmod(pi * 64, 128)
                    nc.vector.tensor_copy(out=fT[slab][row : row + 64, bsl], in_=p_ps[:])

            # ---- MLP ----
            h1s = []
            for m in range(2):
                h1_ps = pms.tile([128, BC], F32, tag="pmisc", name=f"h1ps{m}")
                for kk in range(3):
                    kp = 128 if kk < 2 else 64
                    nc.tensor.matmul(
                        h1_ps[:],
                        lhsT=w1ts[kk][:kp, m * 128 : (m + 1) * 128],
                        rhs=fT[kk][:],
                        start=(kk == 0),
                        stop=(kk == 2),
                    )
                h1 = mlpp.tile([128, BC], F32, tag="h1", name=f"h1_{m}")
                nc.scalar.activation(
                    out=h1[:], in_=h1_ps[:],
                    func=mybir.ActivationFunctionType.Relu, bias=b1s[m][:, 0:1],
                )
                h1s.append(h1)
            h2_ps = pms.tile([128, BC], F32, tag="pmisc", name="h2ps")
            for m in range(2):
                nc.tensor.matmul(
                    h2_ps[:], lhsT=w2ts[m][:], rhs=h1s[m][:],
                    start=(m == 0), stop=(m == 1),
                )
            h2 = mlpp.tile([128, BC], F32, tag="h2", name="h2")
            nc.scalar.activation(
                out=h2[:], in_=h2_ps[:],
                func=mybir.ActivationFunctionType.Relu, bias=b2s[:, 0:1],
            )
            lg_ps = pms.tile([1, BC], F32, tag="pmisc", name="lgps")
            nc.tensor.matmul(lg_ps[:], lhsT=w3ts[:], rhs=h2[:])
            lgt = mlpp.tile([1, BC], F32, tag="lg", name="lgt")
            nc.vector.tensor_scalar_add(lgt[:], lg_ps[:], b3s[:, 0:1])
            nc.sync.dma_start(out=out[None, :], in_=lgt[:])

    nc.compile()
    return nc


def _prep(inp):
    """Host-side input transforms -> per-core in_maps."""
    f32 = np.float32
    bf16 = mybir.dt.np(BF16)
    g = lambda k: np.asarray(inp[k])

    it0 = g("item_tab0").astype(f32)  # [100000, 32]
    it1 = g("item_tab1").astype(f32)  # [10000, 32]
    # bf16 adgroup table, rows padded to 128 elems, sharded into 4x32768 rows
    tab0b = np.zeros((NSH_A * SHA, 128), bf16)
    tab0b[:V0 - 1, :D] = it0.astype(bf16)
    tab0f = np.vstack([it0, np.zeros((V0 - it0.shape[0], D), f32)])
    tab1f = np.vstack([it1, np.zeros((V1 - it1.shape[0], D), f32)])
    tab1b = np.zeros((V1P, D), bf16)
    tab1b[:it1.shape[0]] = it1.astype(bf16)
    mem01 = np.concatenate([g("mem0"), g("mem1")], axis=1).astype(f32)  # [200000,128]
    mem01b = np.zeros((NSH_G * SHG, 128), bf16)
    mem01b[:VM] = mem01.astype(bf16)
    ut0_ = np.ascontiguousarray(g("user_tab0").astype(f32))
    ut1_ = np.ascontiguousarray(g("user_tab1").astype(f32))

    wstack = np.zeros((128, 128), bf16)
    wstack[:G, :G] = g("W_agg0").T.astype(bf16)
    wstack[G:, G:] = g("W_agg1").T.astype(bf16)
    bstack = np.concatenate([g("b_agg0"), g("b_agg1")]).astype(f32)
    w1t = np.ascontiguousarray(g("W1").T.astype(f32))
    w2t = np.ascontiguousarray(g("W2").T.astype(f32))
    w3t = np.ascontiguousarray(g("W3").T.astype(f32))
    b1 = g("b1").astype(f32); b2 = g("b2").astype(f32); b3 = g("b3").astype(f32)

    aseq = g("adgroup_id_seq").astype(np.int64)
    cseq = g("cate_id_seq").astype(np.int64)
    nbr = g("neighbor_ids").astype(np.int64)
    seq_mask = aseq != 0
    invseq_all = (1.0 / np.maximum(seq_mask.sum(-1), 1)).astype(f32)
    nmask = nbr != 0
    invn = (0.5 / np.maximum(nmask.sum(-1), 1)).astype(f32)

    def pack16(stream):
        # [L] -> [128, L//16]: idx k at [k%16, k//16], replicated x8
        w = stream.reshape(-1, 16).T.astype(np.int16)
        return np.tile(w, (8, 1))

    in_maps = []
    for c in range(NC):
        bs = slice(c * BC, (c + 1) * BC)
        a_c, c_c, m_c = aseq[bs], cseq[bs], seq_mask[bs]
        n_c, nm_c = nbr[bs], nmask[bs]
        invn_c = invn[bs]

        aidx_l = np.zeros((NBT * NSH_A * 128, LA // 16), np.int16)
        asel_l = np.zeros((NBT * NSH_A * 128, TA * 32), bf16)
        gidx_l = np.zeros((NBT * NSH_G * 128, LG // 16), np.int16)
        gsel_l = np.zeros((NBT * NSH_G * 128, TG * 128), bf16)
        as_idx_l = np.zeros(NBT * 128, np.int32)
        as_sel_l = np.zeros((NBT * 128, 128), bf16)
        gs_idx_l = np.zeros(NBT * 128, np.int32)
        gs_sel_l = np.zeros((NBT * 128, 128), bf16)
        counts_l = np.zeros((NBT * 79 * 128, 128), bf16)

        for bt in range(NBT):
            btsl = slice(bt * 128, (bt + 1) * 128)
            a = a_c[btsl]; cc = c_c[btsl]; mm = m_c[btsl]
            # flattened (b, s) order, masked
            b_loc = np.repeat(np.arange(128), S)
            av = a.ravel(); mv = mm.ravel()
            b_m = b_loc[mv]; a_m = av[mv]
            sh_a = a_m // SHA; loc_a = a_m % SHA
            spill = []  # (b_loc, global_idx)
            for sh in range(NSH_A):
                pick = sh_a == sh
                bb, ll, gg_ = b_m[pick], loc_a[pick], a_m[pick]
                stream = np.zeros(LA, np.int64)
                selpos, selcol = [], []
                for w in range(NW):
                    inw = (bb // 32) == w
                    lw, bw, gw = ll[inw], bb[inw], gg_[inw]
                    if len(lw) > LW:
                        for z in range(LW, len(lw)):
                            spill.append((bw[z], gw[z]))
                        lw, bw = lw[:LW], bw[:LW]
                    base = w * LW
                    stream[base : base + len(lw)] = lw
                    selpos.append(base + np.arange(len(lw)))
                    selcol.append(bw - 32 * w)
                r0 = (bt * NSH_A + sh) * 128
                aidx_l[r0 : r0 + 128] = pack16(stream)
                sp = np.concatenate(selpos); sc = np.concatenate(selcol)
                tj, pp = sp // 128, sp % 128
                sel = np.zeros((128, TA * 32), f32)
                sel[pp, tj * 32 + sc] = 1.0
                asel_l[r0 : r0 + 128] = sel.astype(bf16)
            assert len(spill) <= 128, f"adgroup spill overflow {len(spill)}"
            for z, (bb, gg_) in enumerate(spill):
                as_idx_l[bt * 128 + z] = gg_
                as_sel_l[bt * 128 + z, bb] = 1.0

            # cate counts [79*128, 128]
            cm = cc.ravel()[mv]
            C = np.bincount(cm * 128 + b_m, minlength=V1P * 128).reshape(V1P, 128)
            counts_l[bt * 79 * 128 : (bt + 1) * 79 * 128] = C.astype(bf16)

            # GNN streams
            nb = n_c[btsl]  # [128, 64]
            msc = (nm_c[btsl].astype(f32) * invn_c[btsl][:, None])  # [128, 64]
            b_loc2 = np.repeat(np.arange(128), N)
            nv = nb.ravel(); mscv = msc.ravel()
            sh_g = nv // SHG; loc_g = nv % SHG
            gspill = []
            for sh in range(NSH_G):
                pick = sh_g == sh
                bb, ll, gg_, ms = b_loc2[pick], loc_g[pick], nv[pick], mscv[pick]
                if len(ll) > LG:
                    for z in range(LG, len(ll)):
                        gspill.append((bb[z], gg_[z], ms[z]))
                    bb, ll, ms = bb[:LG], ll[:LG], ms[:LG]
                stream = np.zeros(LG, np.int64)
                stream[: len(ll)] = ll
                r0 = (bt * NSH_G + sh) * 128
                gidx_l[r0 : r0 + 128] = pack16(stream)
                sp = np.arange(len(ll))
                tj, pp = sp // 128, sp % 128
                sel = np.zeros((128, TG * 128), f32)
                sel[pp, tj * 128 + bb] = ms
                gsel_l[r0 : r0 + 128] = sel.astype(bf16)
            assert len(gspill) <= 128, f"gnn spill overflow {len(gspill)}"
            for z, (bb, gg_, ms) in enumerate(gspill):
                gs_idx_l[bt * 128 + z] = gg_
                gs_sel_l[bt * 128 + z, bb] = ms

        in_maps.append(
            {
                "tab0b": tab0b, "mem01b": mem01b, "tab0f": tab0f, "tab1f": tab1f,
                "ut0": ut0_, "ut1": ut1_, "tab1b": tab1b, "counts": counts_l,
                "iu0": g("user_f0")[bs].astype(np.int32),
                "iu1": g("user_f1")[bs].astype(np.int32),
                "iad": g("adgroup_id")[bs].astype(np.int32),
                "icat": g("cate_id")[bs].astype(np.int32),
                "aidx": aidx_l, "gidx": gidx_l, "asel": asel_l, "gsel": gsel_l,
                "as_idx": as_idx_l, "as_sel": as_sel_l,
                "gs_idx": gs_idx_l, "gs_sel": gs_sel_l,
                "invseq": invseq_all[bs].reshape(NBT, 128).T.copy(),
                "wstack": wstack, "bstack": bstack,
                "w1t": w1t, "b1": b1, "w2t": w2t, "b2": b2, "w3t": w3t, "b3": b3,
            }
        )
    return in_maps


def kernel(**inputs) -> np.ndarray:
    if "nc" not in _CACHE:
        _CACHE["nc"] = _build()
    nc = _CACHE["nc"]
    in_maps = _prep(inputs)
    trace = bool(_os.environ.get("KERNEL_TRACE"))
    res = run_bass_kernel_spmd(nc, in_maps, list(range(NC)), trace=trace)
    _CACHE["last_result"] = res
    out = np.concatenate([res.results[c]["out"] for c in range(NC)])
    return out[:, None].astype(np.float32)

